# revision 2
# baseline (speedup 1.0000x reference)
import os
import sys
sys.path.insert(0, '/opt/trn_rl_repo')
import numpy as np
import concourse.bass as bass
import concourse.bacc as bacc
import concourse.mybir as mybir
import concourse.tile as tile
from concourse.bass import IndirectOffsetOnAxis
from concourse.bass_utils import run_bass_kernel_spmd

P = 128
T = 1024
S = 1024
D = 512
H = 8
DK = 64
DFF = 2048
VOC = 32000
NT = T // P   # 8 token tiles
ND = D // P   # 4 d-model chunks
NJ = DFF // P  # 16 dff tiles
L_FULL = 6
EPS = 1e-5
NEG = -1e9

F32 = mybir.dt.float32
F32R = mybir.dt.float32r
I32 = mybir.dt.int32
AF = mybir.ActivationFunctionType
OP = mybir.AluOpType


def _pe_table():
    pos = np.arange(T)[:, None].astype(np.float64)
    div = np.exp(np.arange(0, D, 2).astype(np.float64) * (-np.log(10000.0) / D))
    pe = np.zeros((T, D))
    pe[:, 0::2] = np.sin(pos * div)
    pe[:, 1::2] = np.cos(pos * div)
    return pe.astype(np.float32)


def _causal_quads():
    # [P, 4, 512]: mask NEG where (128*r + kk) > qq else 0 (kk = partition)
    kk = np.arange(P)[:, None]
    qq = np.arange(512)[None, :]
    out = np.zeros((P, 4, 512), np.float32)
    for r in range(4):
        out[:, r, :] = np.where(128 * r + kk > qq, NEG, 0.0)
    return out


def build(n_layers=L_FULL, repeat=1):
    nc = bacc.Bacc("TRN2", target_bir_lowering=False, debug=False, num_devices=8)

    embd = nc.dram_tensor("emb", [VOC, D], F32, kind="ExternalInput")
    decd = nc.dram_tensor("dec_idx", [P, NT], I32, kind="ExternalInput")
    encd = nc.dram_tensor("enc_idx", [P, NT], I32, kind="ExternalInput")
    ped = nc.dram_tensor("pe", [T, D], F32, kind="ExternalInput")
    causd = nc.dram_tensor("causal", [P, 4, 512], F32R, kind="ExternalInput")
    identrd = nc.dram_tensor("identr", [P, P], F32R, kind="ExternalInput")
    identd = nc.dram_tensor("ident", [P, P], F32, kind="ExternalInput")
    encxd = nc.dram_tensor("encx", [S, D], F32, kind="ExternalInput")
    WQS = nc.dram_tensor("wq_s", [n_layers, D, D], F32R, kind="ExternalInput")
    WKS = nc.dram_tensor("wk_s", [n_layers, D, D], F32R, kind="ExternalInput")
    WVS = nc.dram_tensor("wv_s", [n_layers, D, D], F32R, kind="ExternalInput")
    WOS = nc.dram_tensor("wo_s", [n_layers, D, D], F32R, kind="ExternalInput")
    WQC = nc.dram_tensor("wq_c", [n_layers, D, D], F32R, kind="ExternalInput")
    WKC = nc.dram_tensor("wk_c", [n_layers, D, D], F32R, kind="ExternalInput")
    WVC = nc.dram_tensor("wv_c", [n_layers, D, D], F32R, kind="ExternalInput")
    WOC = nc.dram_tensor("wo_c", [n_layers, D, D], F32R, kind="ExternalInput")
    W1D = nc.dram_tensor("w1", [n_layers, D, DFF], F32R, kind="ExternalInput")
    W2D = nc.dram_tensor("w2", [n_layers, DFF, D], F32R, kind="ExternalInput")
    outd = nc.dram_tensor("out", [T, D], F32, kind="ExternalOutput")

    with nc.allow_low_precision(reason="f32r rounding intended"), \
         tile.TileContext(nc) as tc:
        with tc.tile_pool(name="pers", bufs=1) as pers, \
             tc.tile_pool(name="dbl", bufs=2) as dbl, \
             tc.tile_pool(name="p3", bufs=3) as p3, \
             tc.tile_pool(name="p2", bufs=2) as p2, \
             tc.tile_pool(name="p5", bufs=5) as p5, \
             tc.tile_pool(name="p4", bufs=4) as p4, \
             tc.tile_pool(name="psS", bufs=2, space="PSUM") as psS, \
             tc.tile_pool(name="psB", bufs=4, space="PSUM") as psB:

            # ---------------- persistent tiles ----------------
            x_res = pers.tile([P, NT, D], F32)       # [tok_in_tile, t_tile, D]
            xT = pers.tile([P, ND, T], F32R)         # [d_in_chunk, d_chunk, tok]
            encT = pers.tile([P, ND, S], F32R)
            QT = pers.tile([P, ND, T], F32R)
            KT = pers.tile([P, ND, T], F32R)
            vext = pers.tile([P, NT, H, DK + 1], F32R)
            causal_sb = pers.tile([P, 4, 512], F32R)
            identr_sb = pers.tile([P, P], F32R)
            ident_sb = pers.tile([P, P], F32)
            ones_f = pers.tile([P, 64], F32)
            ones64 = pers.tile([1, 64], F32R)
            eps_sb = pers.tile([P, 1], F32)
            dec_sb = pers.tile([P, NT], I32)
            enc_sb = pers.tile([P, NT], I32)
            msc_dec = pers.tile([P, NT], F32)
            msc_enc = pers.tile([P, NT], F32)

            nc.sync.dma_start(out=causal_sb, in_=causd[:, :, :])
            nc.sync.dma_start(out=identr_sb, in_=identrd[:, :])
            nc.sync.dma_start(out=ident_sb, in_=identd[:, :])
            nc.sync.dma_start(out=dec_sb, in_=decd[:, :])
            nc.sync.dma_start(out=enc_sb, in_=encd[:, :])
            nc.vector.memset(ones_f, 1.0)
            nc.vector.memset(eps_sb, EPS)
            nc.vector.tensor_copy(out=ones64, in_=ones_f[0:1, :])

            # pad multipliers (0 for pad token, 1 otherwise)
            for tok_sb, msc in ((dec_sb, msc_dec), (enc_sb, msc_enc)):
                tokf = p4.tile([P, NT], F32, tag="tokf")
                nc.vector.tensor_copy(out=tokf, in_=tok_sb)
                is0 = p4.tile([P, NT], F32, tag="is0")
                nc.vector.tensor_scalar(out=is0, in0=tokf, scalar1=0.0,
                                        scalar2=None, op0=OP.is_equal)
                nc.scalar.activation(out=msc, in_=is0, func=AF.Copy,
                                     bias=1.0, scale=-1.0)

            # ---------------- embedding + pe ----------------
            for tt in range(NT):
                g = p3.tile([P, D], F32, tag="tmp")
                nc.gpsimd.indirect_dma_start(
                    out=g, out_offset=None, in_=embd[:, :],
                    in_offset=IndirectOffsetOnAxis(ap=dec_sb[:, tt:tt + 1], axis=0))
                pe_t = p3.tile([P, D], F32, tag="tmp")
                nc.sync.dma_start(out=pe_t, in_=ped[tt * P:(tt + 1) * P, :])
                g2 = p3.tile([P, D], F32, tag="tmp")
                nc.vector.tensor_scalar(out=g2, in0=g, scalar1=msc_dec[:, tt:tt + 1],
                                        scalar2=None, op0=OP.mult)
                nc.vector.tensor_add(out=x_res[:, tt, :], in0=g2, in1=pe_t)

            TRANSPOSE_BATCH = os.environ.get("KBATCH_T", "0") == "1"

            def transpose_to(dst, src_of_tt, tts):
                # dst [P, ND, T] f32r; src_of_tt(tt) -> [P, D] f32 AP.
                # tts must be consecutive, length multiple of 4.
                for d in range(ND):
                    if TRANSPOSE_BATCH:
                        for g0 in range(0, len(tts), 4):
                            grp = tts[g0:g0 + 4]
                            ps_t = psB.tile([P, 512], F32, tag="b",
                                            name=f"pst_{d}_{grp[0]}")
                            for qi, tt in enumerate(grp):
                                nc.tensor.matmul(
                                    ps_t[:, qi * P:(qi + 1) * P],
                                    src_of_tt(tt)[:, d * P:(d + 1) * P],
                                    ident_sb, is_transpose=True,
                                    start=(qi == 0), stop=(qi == 3),
                                    skip_group_check=True)
                            nc.vector.tensor_copy(
                                out=dst[:, d, grp[0] * P:(grp[0] + 4) * P],
                                in_=ps_t)
                    else:
                        for tt in tts:
                            ps_t = psB.tile([P, P], F32, tag="b",
                                            name=f"pst_{d}_{tt}")
                            nc.tensor.transpose(
                                out=ps_t,
                                in_=src_of_tt(tt)[:, d * P:(d + 1) * P],
                                identity=ident_sb)
                            nc.vector.tensor_copy(
                                out=dst[:, d, tt * P:(tt + 1) * P], in_=ps_t)

            transpose_to(xT, lambda tt: x_res[:, tt, :], list(range(NT)))

            for g0 in range(0, NT, 4):
                e_ts = []
                for tt in range(g0, g0 + 4):
                    e_t = p4.tile([P, D], F32, tag="enc", name=f"enc_{tt}")
                    nc.sync.dma_start(out=e_t, in_=encxd[tt * P:(tt + 1) * P, :])
                    e_ts.append(e_t)
                transpose_to(encT, lambda tt: e_ts[tt - g0],
                             list(range(g0, g0 + 4)))

            # ---------------- helpers ----------------
            def ln_into_xres(ps_in, tt):
                pre = p3.tile([P, D], F32, tag="tmp")
                nc.vector.tensor_add(out=pre, in0=ps_in, in1=x_res[:, tt, :])
                st = p4.tile([P, nc.vector.BN_STATS_DIM], F32, tag="st")
                nc.vector.bn_stats(out=st, in_=pre)
                mv = p4.tile([P, nc.vector.BN_AGGR_DIM], F32, tag="mv")
                nc.vector.bn_aggr(out=mv, in_=st)
                std = p4.tile([P, 1], F32, tag="sd")
                nc.scalar.activation(out=std, in_=mv[:, 1:2], func=AF.Sqrt,
                                     bias=eps_sb, scale=1.0)
                rstd = p4.tile([P, 1], F32, tag="rs")
                nc.vector.reciprocal(out=rstd, in_=std)
                nc.vector.tensor_scalar(out=x_res[:, tt, :], in0=pre,
                                        scalar1=mv[:, 0:1], scalar2=rstd,
                                        op0=OP.subtract, op1=OP.mult)

            def load_wattn(wd, l):
                w = dbl.tile([P, ND, D], F32R, tag="wattn")
                nc.gpsimd.dma_start(
                    out=w, in_=wd[l].rearrange("(kc kp) n -> kp kc n", kp=P))
                return w

            def attn(l, is_self):
                wq = load_wattn(WQS if is_self else WQC, l)
                wk = load_wattn(WKS if is_self else WKC, l)
                wv = load_wattn(WVS if is_self else WVC, l)
                wo = load_wattn(WOS if is_self else WOC, l)
                kv = xT if is_self else encT
                msc = msc_dec if is_self else msc_enc

                # QT / KT projections
                for dst, w, src in ((QT, wq, xT), (KT, wk, kv)):
                    for dq in range(ND):
                        for c in range(2):
                            ps = psS.tile([P, 512], F32, tag="s")
                            for kc in range(ND):
                                nc.tensor.matmul(
                                    ps, w[:, kc, dq * P:(dq + 1) * P],
                                    src[:, kc, c * 512:(c + 1) * 512],
                                    start=(kc == 0), stop=(kc == ND - 1))
                            nc.any.tensor_copy(
                                out=dst[:, dq, c * 512:(c + 1) * 512], in_=ps)

                # V projection, pad rows zeroed via msc
                for i in range(NT):
                    ps = psS.tile([P, 512], F32, tag="s")
                    for kc in range(ND):
                        nc.tensor.matmul(ps, kv[:, kc, i * P:(i + 1) * P],
                                         wv[:, kc, :],
                                         start=(kc == 0), stop=(kc == ND - 1))
                    nc.scalar.activation(
                        out=vext[:, i, :, 0:DK],
                        in_=ps.rearrange("p (h v) -> p h v", h=H),
                        func=AF.Copy, scale=msc[:, i:i + 1])
                # ones column (also zeroed on pad rows)
                for h in range(H):
                    nc.scalar.activation(
                        out=vext[:, :, h, DK:DK + 1],
                        in_=msc.rearrange("p (t o) -> p t o", o=1),
                        func=AF.Copy)

                # scores -> exp -> AV (k-tiles processed in pairs)
                for c in range(2):
                    ctx_pairs = [p5.tile([P, 512], F32R, tag="ctx",
                                         name=f"ctxp_{l}_{is_self}_{c}_{d}")
                                 for d in range(ND)]
                    for d in range(ND):
                        for hh in range(2):
                            h = 2 * d + hh
                            hsl = slice(hh * 64, (hh + 1) * 64)
                            kmax = 4 * (c + 1) if is_self else NT
                            ps_ctx = psB.tile([DK + 1, 512], F32, tag="b")
                            for i0 in range(0, kmax, 2):
                                ps_s = psS.tile([P, 1024], F32, tag="s")
                                diag = is_self and i0 >= 4 * c
                                for half, i in ((0, i0), (1, i0 + 1)):
                                    sl_ = slice(half * 512, (half + 1) * 512)
                                    if diag:
                                        r = i - 4 * c
                                        nc.tensor.matmul(
                                            ps_s[:, sl_], identr_sb,
                                            causal_sb[:, r, :],
                                            start=True, stop=False,
                                            skip_group_check=True)
                                    nc.tensor.matmul(
                                        ps_s[:, sl_],
                                        KT[hsl, d, i * P:(i + 1) * P],
                                        QT[hsl, d, c * 512:(c + 1) * 512],
                                        start=not diag, stop=True,
                                        skip_group_check=True)
                                e = p2.tile([P, 1024], F32R, tag="exp")
                                nc.scalar.activation(out=e, in_=ps_s,
                                                     func=AF.Exp, scale=0.125)
                                for half, i in ((0, i0), (1, i0 + 1)):
                                    nc.tensor.matmul(
                                        ps_ctx, vext[:, i, h, :],
                                        e[:, half * 512:(half + 1) * 512],
                                        start=(i == 0), stop=(i == kmax - 1))
                            recip = p3.tile([1, 512], F32R, tag="recip")
                            nc.vector.reciprocal(out=recip,
                                                 in_=ps_ctx[DK:DK + 1, :])
                            ps_r = psB.tile([64, 512], F32, tag="b")
                            nc.tensor.matmul(ps_r, ones64, recip,
                                             start=True, stop=True)
                            ctxe = p3.tile([64, 512], F32, tag="tmp")
                            nc.vector.tensor_copy(out=ctxe, in_=ps_ctx[0:DK, :])
                            nc.vector.tensor_mul(out=ctx_pairs[d][hsl, :],
                                                 in0=ctxe, in1=ps_r)
                    # output projection + residual + LN for this chunk
                    for ts_ in range(4):
                        tt = 4 * c + ts_
                        ps_o = psB.tile([P, 512], F32, tag="b")
                        for d in range(ND):
                            nc.tensor.matmul(
                                ps_o, ctx_pairs[d][:, ts_ * P:(ts_ + 1) * P],
                                wo[:, d, :], start=(d == 0), stop=(d == ND - 1))
                        ln_into_xres(ps_o, tt)
                transpose_to(xT, lambda tt: x_res[:, tt, :], list(range(NT)))

            def ffn(l, last):
                for c in range(2):
                    ps_fs = [psB.tile([P, 512], F32, tag="b",
                                      name=f"psf_{l}_{c}_{i}") for i in range(4)]
                    for j in range(NJ):
                        if j % 4 == 0:
                            jc = j // 4
                            w1t = dbl.tile([P, ND, 512], F32R, tag="w1c")
                            nc.gpsimd.dma_start(
                                out=w1t,
                                in_=W1D[l][:, jc * 512:(jc + 1) * 512]
                                .rearrange("(kc kp) n -> kp kc n", kp=P))
                            w2t = dbl.tile([P, 4, D], F32R, tag="w2c")
                            nc.gpsimd.dma_start(
                                out=w2t,
                                in_=W2D[l][jc * 512:(jc + 1) * 512, :]
                                .rearrange("(jj kp) n -> kp jj n", kp=P))
                        ps_h = psS.tile([P, 512], F32, tag="s")
                        for kc in range(ND):
                            nc.tensor.matmul(
                                ps_h, w1t[:, kc, (j % 4) * P:(j % 4 + 1) * P],
                                xT[:, kc, c * 512:(c + 1) * 512],
                                start=(kc == 0), stop=(kc == ND - 1))
                        hT = p3.tile([P, 512], F32R, tag="hT")
                        nc.scalar.activation(out=hT, in_=ps_h, func=AF.Relu)
                        for ts_ in range(4):
                            nc.tensor.matmul(
                                ps_fs[ts_], hT[:, ts_ * P:(ts_ + 1) * P],
                                w2t[:, j % 4, :],
                                start=(j == 0), stop=(j == NJ - 1))
                    for ts_ in range(4):
                        ln_into_xres(ps_fs[ts_], 4 * c + ts_)
                    if not last:
                        transpose_to(xT, lambda tt: x_res[:, tt, :],
                                     list(range(4 * c, 4 * c + 4)))

            # ---------------- layers ----------------
            for rep in range(repeat):
                for l in range(n_layers):
                    attn(l, True)
                    attn(l, False)
                    ffn(l, last=(rep == repeat - 1 and l == n_layers - 1))

            for tt in range(NT):
                nc.sync.dma_start(out=outd[tt * P:(tt + 1) * P, :],
                                  in_=x_res[:, tt, :])

    nc.compile()
    return nc


_CACHE = {}


def get_nc(n_layers=L_FULL, repeat=1):
    key = (n_layers, repeat)
    if key not in _CACHE:
        _CACHE[key] = build(n_layers, repeat)
    return _CACHE[key]


def make_in_maps(dec_inputs, enc_inputs, enc_outputs, emb,
                 Wq_self, Wk_self, Wv_self, Wo_self,
                 Wq_cross, Wk_cross, Wv_cross, Wo_cross, W1, W2,
                 n_layers=L_FULL):
    f = np.ascontiguousarray
    emb = f(np.asarray(emb, dtype=np.float32))
    dec = np.asarray(dec_inputs).astype(np.int32)
    enc = np.asarray(enc_inputs).astype(np.int32)
    encx = np.asarray(enc_outputs, dtype=np.float32)
    pe = _pe_table()
    caus = _causal_quads()
    ident = np.eye(P, dtype=np.float32)
    ws = {}
    for name, w in (("wq_s", Wq_self), ("wk_s", Wk_self), ("wv_s", Wv_self),
                    ("wo_s", Wo_self), ("wq_c", Wq_cross), ("wk_c", Wk_cross),
                    ("wv_c", Wv_cross), ("wo_c", Wo_cross), ("w1", W1),
                    ("w2", W2)):
        ws[name] = f(np.asarray(w, dtype=np.float32)[:n_layers])
    B = dec.shape[0]
    in_maps = []
    for b in range(B):
        m = dict(emb=emb, pe=pe, causal=caus, ident=ident, identr=ident,
                 dec_idx=f(dec[b].reshape(NT, P).T),
                 enc_idx=f(enc[b].reshape(NT, P).T),
                 encx=f(encx[b]), **ws)
        in_maps.append(m)
    return in_maps


LAST_RESULT = None


def kernel(**inputs):
    global LAST_RESULT
    n_layers = inputs.pop("_n_layers", L_FULL)
    trace = inputs.pop("_trace", False)
    tmpdir = inputs.pop("_tmpdir", None)
    nc = get_nc(n_layers)
    in_maps = make_in_maps(**inputs, n_layers=n_layers)
    res = run_bass_kernel_spmd(nc, in_maps, core_ids=list(range(len(in_maps))),
                               trace=trace, tmpdir=tmpdir)
    LAST_RESULT = res
    out = np.stack([r["out"] for r in res.results], axis=0)
    return out



# revision 18
# speedup vs baseline: 1.3798x; 1.3798x over previous
import os
import sys
sys.path.insert(0, '/opt/trn_rl_repo')
import numpy as np
import ml_dtypes
import concourse.bass as bass
import concourse.bacc as bacc
import concourse.mybir as mybir
import concourse.tile as tile
from concourse.bass import IndirectOffsetOnAxis
from concourse.bass_utils import run_bass_kernel_spmd

P = 128
T = 1024
S = 1024
D = 512
H = 8
DK = 64
DFF = 2048
VOC = 32000
NT = T // P   # 8 token tiles
ND = D // P   # 4 d-model chunks
NJ = DFF // P  # 16 dff tiles
L_FULL = 6
EPS = 1e-5
NEG = -1e9

F32 = mybir.dt.float32
BF16 = mybir.dt.bfloat16
I32 = mybir.dt.int32
AF = mybir.ActivationFunctionType
OP = mybir.AluOpType
BF = ml_dtypes.bfloat16


def _pe_table():
    pos = np.arange(T)[:, None].astype(np.float64)
    div = np.exp(np.arange(0, D, 2).astype(np.float64) * (-np.log(10000.0) / D))
    pe = np.zeros((T, D))
    pe[:, 0::2] = np.sin(pos * div)
    pe[:, 1::2] = np.cos(pos * div)
    return pe.astype(np.float32)


def _causal_quads():
    # [P, 4, 512]: mask NEG where (128*r + kk) > qq else 0 (kk = partition)
    kk = np.arange(P)[:, None]
    qq = np.arange(512)[None, :]
    out = np.zeros((P, 4, 512), np.float32)
    for r in range(4):
        out[:, r, :] = np.where(128 * r + kk > qq, NEG, 0.0)
    return out


def build(n_layers=L_FULL, repeat=1):
    DBG = os.environ.get("KDBG", "0") == "1"
    nc = bacc.Bacc("TRN2", target_bir_lowering=False, debug=False, num_devices=8)

    embd = nc.dram_tensor("emb", [VOC, D], F32, kind="ExternalInput")
    decd = nc.dram_tensor("dec_idx", [P, NT], I32, kind="ExternalInput")
    encd = nc.dram_tensor("enc_idx", [P, NT], I32, kind="ExternalInput")
    ped = nc.dram_tensor("pe", [T, D], F32, kind="ExternalInput")
    causd = nc.dram_tensor("causal", [P, 4, 512], BF16, kind="ExternalInput")
    identrd = nc.dram_tensor("identr", [P, P], BF16, kind="ExternalInput")
    identd = nc.dram_tensor("ident", [P, P], F32, kind="ExternalInput")
    encxd = nc.dram_tensor("encx", [S, D], F32, kind="ExternalInput")
    WQS = nc.dram_tensor("wq_s", [n_layers, D, D], BF16, kind="ExternalInput")
    WKS = nc.dram_tensor("wk_s", [n_layers, D, D], BF16, kind="ExternalInput")
    WVS = nc.dram_tensor("wv_s", [n_layers, D, D], BF16, kind="ExternalInput")
    WOS = nc.dram_tensor("wo_s", [n_layers, D, D], BF16, kind="ExternalInput")
    WQC = nc.dram_tensor("wq_c", [n_layers, D, D], BF16, kind="ExternalInput")
    WKC = nc.dram_tensor("wk_c", [n_layers, D, D], BF16, kind="ExternalInput")
    WVC = nc.dram_tensor("wv_c", [n_layers, D, D], BF16, kind="ExternalInput")
    WOC = nc.dram_tensor("wo_c", [n_layers, D, D], BF16, kind="ExternalInput")
    W1D = nc.dram_tensor("w1", [n_layers, D, DFF], BF16, kind="ExternalInput")
    W2D = nc.dram_tensor("w2", [n_layers, DFF, D], BF16, kind="ExternalInput")
    outd = nc.dram_tensor("out", [T, D], F32, kind="ExternalOutput")
    if DBG:
        dbg = {
            "dbg_x0": nc.dram_tensor("dbg_x0", [P, NT * D], F32,
                                     kind="ExternalOutput"),
            "dbg_xT": nc.dram_tensor("dbg_xT", [P, ND * T], BF16,
                                     kind="ExternalOutput"),
            "dbg_QT": nc.dram_tensor("dbg_QT", [P, ND * T], BF16,
                                     kind="ExternalOutput"),
            "dbg_KT": nc.dram_tensor("dbg_KT", [P, ND * T], BF16,
                                     kind="ExternalOutput"),
            "dbg_vext": nc.dram_tensor("dbg_vext", [P, NT * H * P],
                                       BF16, kind="ExternalOutput"),
            "dbg_e": nc.dram_tensor("dbg_e", [P, 1024], BF16,
                                    kind="ExternalOutput"),
            "dbg_recip": nc.dram_tensor("dbg_recip", [1, 512], F32,
                                        kind="ExternalOutput"),
            "dbg_rb": nc.dram_tensor("dbg_rb", [DK, 512], F32,
                                     kind="ExternalOutput"),
            "dbg_ctx": nc.dram_tensor("dbg_ctx", [P, 512], BF16,
                                      kind="ExternalOutput"),
            "dbg_xa": nc.dram_tensor("dbg_xa", [P, NT * D], F32,
                                     kind="ExternalOutput"),
        }

    with nc.allow_low_precision(reason="bf16 matmuls intended"), \
         tile.TileContext(nc) as tc:
        with tc.tile_pool(name="pers", bufs=1) as pers, \
             tc.tile_pool(name="dbl", bufs=2) as dbl, \
             tc.tile_pool(name="p3", bufs=3) as p3, \
             tc.tile_pool(name="p2", bufs=2) as p2, \
             tc.tile_pool(name="p5", bufs=5) as p5, \
             tc.tile_pool(name="p4", bufs=4) as p4, \
             tc.tile_pool(name="psS", bufs=2, space="PSUM") as psS, \
             tc.tile_pool(name="psB", bufs=4, space="PSUM") as psB:

            # ---------------- persistent tiles ----------------
            x_res = pers.tile([P, NT, D], F32)       # [tok_in_tile, t_tile, D]
            xT = pers.tile([P, ND, T], BF16)         # [d_in_chunk, d_chunk, tok]
            encT = pers.tile([P, ND, S], BF16)
            QT = pers.tile([P, ND, T], BF16)
            KT = pers.tile([P, ND, T], BF16)
            # per (tile, head): col 0 = ones (pad-masked), cols 64..127 = V
            vext = pers.tile([P, NT, H, P], BF16)
            causal_sb = pers.tile([P, 4, 512], BF16)
            identr_sb = pers.tile([P, P], BF16)
            ident_sb = pers.tile([P, P], F32)
            eps_sb = pers.tile([P, 1], F32)
            dec_sb = pers.tile([P, NT], I32)
            enc_sb = pers.tile([P, NT], I32)
            msc_dec = pers.tile([P, NT], F32)
            msc_enc = pers.tile([P, NT], F32)

            nc.sync.dma_start(out=causal_sb, in_=causd[:, :, :])
            nc.sync.dma_start(out=identr_sb, in_=identrd[:, :])
            nc.sync.dma_start(out=ident_sb, in_=identd[:, :])
            nc.sync.dma_start(out=dec_sb, in_=decd[:, :])
            nc.sync.dma_start(out=enc_sb, in_=encd[:, :])
            nc.vector.memset(eps_sb, EPS)
            nc.vector.memset(vext, 0.0)

            # pad multipliers (0 for pad token, 1 otherwise)
            for tok_sb, msc in ((dec_sb, msc_dec), (enc_sb, msc_enc)):
                tokf = p4.tile([P, NT], F32, tag="tokf")
                nc.vector.tensor_copy(out=tokf, in_=tok_sb)
                is0 = p4.tile([P, NT], F32, tag="is0")
                nc.vector.tensor_scalar(out=is0, in0=tokf, scalar1=0.0,
                                        scalar2=None, op0=OP.is_equal)
                nc.scalar.activation(out=msc, in_=is0, func=AF.Copy,
                                     bias=1.0, scale=-1.0)

            # ---------------- embedding + pe ----------------
            for tt in range(NT):
                g = p3.tile([P, D], F32, tag="tmp")
                nc.gpsimd.indirect_dma_start(
                    out=g, out_offset=None, in_=embd[:, :],
                    in_offset=IndirectOffsetOnAxis(ap=dec_sb[:, tt:tt + 1], axis=0))
                pe_t = p3.tile([P, D], F32, tag="tmp")
                nc.sync.dma_start(out=pe_t, in_=ped[tt * P:(tt + 1) * P, :])
                g2 = p3.tile([P, D], F32, tag="tmp")
                nc.vector.tensor_scalar(out=g2, in0=g, scalar1=msc_dec[:, tt:tt + 1],
                                        scalar2=None, op0=OP.mult)
                nc.vector.tensor_add(out=x_res[:, tt, :], in0=g2, in1=pe_t)

            def transpose_to(dst, src_of_tt, tts):
                # dst [P, ND, T] bf16; src_of_tt(tt) -> [P, D] f32 AP.
                for d in range(ND):
                    for tt in tts:
                        ps_t = psB.tile([P, P], F32, tag="b",
                                        name=f"pst_{d}_{tt}")
                        nc.tensor.transpose(
                            out=ps_t,
                            in_=src_of_tt(tt)[:, d * P:(d + 1) * P],
                            identity=ident_sb)
                        nc.vector.tensor_copy(
                            out=dst[:, d, tt * P:(tt + 1) * P], in_=ps_t)

            transpose_to(xT, lambda tt: x_res[:, tt, :], list(range(NT)))
            if DBG:
                nc.sync.dma_start(out=dbg["dbg_x0"][:, :],
                                  in_=x_res.rearrange("p t d -> p (t d)"))
                nc.sync.dma_start(out=dbg["dbg_xT"][:, :],
                                  in_=xT.rearrange("p c t -> p (c t)"))

            for g0 in range(0, NT, 4):
                e_ts = []
                for tt in range(g0, g0 + 4):
                    e_t = p4.tile([P, D], F32, tag="enc", name=f"enc_{tt}")
                    nc.sync.dma_start(out=e_t, in_=encxd[tt * P:(tt + 1) * P, :])
                    e_ts.append(e_t)
                transpose_to(encT, lambda tt: e_ts[tt - g0],
                             list(range(g0, g0 + 4)))

            # ---------------- helpers ----------------
            def ln_into_xres(ps_in, tt):
                pre = p3.tile([P, D], F32, tag="tmp")
                nc.vector.tensor_add(out=pre, in0=ps_in, in1=x_res[:, tt, :])
                st = p4.tile([P, nc.vector.BN_STATS_DIM], F32, tag="st")
                nc.vector.bn_stats(out=st, in_=pre)
                mv = p4.tile([P, nc.vector.BN_AGGR_DIM], F32, tag="mv")
                nc.vector.bn_aggr(out=mv, in_=st)
                std = p4.tile([P, 1], F32, tag="sd")
                nc.scalar.activation(out=std, in_=mv[:, 1:2], func=AF.Sqrt,
                                     bias=eps_sb, scale=1.0)
                rstd = p4.tile([P, 1], F32, tag="rs")
                nc.vector.reciprocal(out=rstd, in_=std)
                nc.vector.tensor_scalar(out=x_res[:, tt, :], in0=pre,
                                        scalar1=mv[:, 0:1], scalar2=rstd,
                                        op0=OP.subtract, op1=OP.mult)

            def load_wattn(wd, l):
                w = dbl.tile([P, ND, D], BF16, tag="wattn")
                nc.gpsimd.dma_start(
                    out=w, in_=wd[l].rearrange("(kc kp) n -> kp kc n", kp=P))
                return w

            def attn(l, is_self):
                wq = load_wattn(WQS if is_self else WQC, l)
                wk = load_wattn(WKS if is_self else WKC, l)
                wv = load_wattn(WVS if is_self else WVC, l)
                wo = load_wattn(WOS if is_self else WOC, l)
                kv = xT if is_self else encT
                msc = msc_dec if is_self else msc_enc

                # QT / KT projections
                for dst, w, src in ((QT, wq, xT), (KT, wk, kv)):
                    for dq in range(ND):
                        for c in range(2):
                            ps = psS.tile([P, 512], F32, tag="s")
                            for kc in range(ND):
                                nc.tensor.matmul(
                                    ps, w[:, kc, dq * P:(dq + 1) * P],
                                    src[:, kc, c * 512:(c + 1) * 512],
                                    start=(kc == 0), stop=(kc == ND - 1))
                            nc.any.tensor_copy(
                                out=dst[:, dq, c * 512:(c + 1) * 512], in_=ps)

                # V projection, pad rows zeroed via msc
                for i in range(NT):
                    ps = psS.tile([P, 512], F32, tag="s")
                    for kc in range(ND):
                        nc.tensor.matmul(ps, kv[:, kc, i * P:(i + 1) * P],
                                         wv[:, kc, :],
                                         start=(kc == 0), stop=(kc == ND - 1))
                    nc.scalar.activation(
                        out=vext[:, i, :, DK:P],
                        in_=ps.rearrange("p (h v) -> p h v", h=H),
                        func=AF.Copy, scale=msc[:, i:i + 1])
                # ones column (also zeroed on pad rows)
                for h in range(H):
                    nc.scalar.activation(
                        out=vext[:, :, h, 0:1],
                        in_=msc.rearrange("p (t o) -> p t o", o=1),
                        func=AF.Copy)
                if DBG and l == 0 and is_self:
                    nc.sync.dma_start(out=dbg["dbg_QT"][:, :],
                                      in_=QT.rearrange("p c t -> p (c t)"))
                    nc.sync.dma_start(out=dbg["dbg_KT"][:, :],
                                      in_=KT.rearrange("p c t -> p (c t)"))
                    nc.sync.dma_start(
                        out=dbg["dbg_vext"][:, :],
                        in_=vext.rearrange("p t h v -> p (t h v)"))

                # scores -> exp -> AV (k-tiles processed in pairs)
                for c in range(2):
                    ctx_pairs = [p5.tile([P, 512], BF16, tag="ctx",
                                         name=f"ctxp_{l}_{is_self}_{c}_{d}")
                                 for d in range(ND)]
                    for d in range(ND):
                        for hh in range(2):
                            h = 2 * d + hh
                            hsl = slice(hh * 64, (hh + 1) * 64)
                            kmax = 4 * (c + 1) if is_self else NT
                            ps_ctx = psB.tile([P, 512], F32, tag="b")
                            for i0 in range(0, kmax, 2):
                                ps_s = psS.tile([P, 1024], F32, tag="s")
                                diag = is_self and i0 >= 4 * c
                                for half, i in ((0, i0), (1, i0 + 1)):
                                    sl_ = slice(half * 512, (half + 1) * 512)
                                    if diag:
                                        r = i - 4 * c
                                        nc.tensor.matmul(
                                            ps_s[:, sl_], identr_sb,
                                            causal_sb[:, r, :],
                                            start=True, stop=False,
                                            skip_group_check=True)
                                    nc.tensor.matmul(
                                        ps_s[:, sl_],
                                        KT[hsl, d, i * P:(i + 1) * P],
                                        QT[hsl, d, c * 512:(c + 1) * 512],
                                        start=not diag, stop=True,
                                        skip_group_check=True)
                                e = p2.tile([P, 1024], BF16, tag="exp")
                                nc.scalar.activation(out=e, in_=ps_s,
                                                     func=AF.Exp, scale=0.125)
                                if (DBG and l == 0 and is_self and c == 0
                                        and d == 0 and hh == 0 and i0 == 0):
                                    nc.sync.dma_start(out=dbg["dbg_e"][:, :], in_=e)
                                for half, i in ((0, i0), (1, i0 + 1)):
                                    nc.tensor.matmul(
                                        ps_ctx, vext[:, i, h, :],
                                        e[:, half * 512:(half + 1) * 512],
                                        start=(i == 0), stop=(i == kmax - 1))
                            recip = p3.tile([1, 512], F32, tag="recip")
                            nc.vector.reciprocal_approx_fast(
                                out=recip, in_=ps_ctx[0:1, :])
                            rb = p3.tile([DK, 512], F32, tag="rb")
                            nc.gpsimd.partition_broadcast(rb, recip)
                            nc.vector.tensor_mul(out=ctx_pairs[d][hsl, :],
                                                 in0=ps_ctx[DK:P, :], in1=rb)
                            if (DBG and l == 0 and is_self and c == 0
                                    and d == 0 and hh == 0):
                                nc.sync.dma_start(out=dbg["dbg_recip"][:, :],
                                                  in_=recip)
                                nc.sync.dma_start(out=dbg["dbg_rb"][:, :], in_=rb)
                    if DBG and l == 0 and is_self and c == 0:
                        nc.sync.dma_start(out=dbg["dbg_ctx"][:, :],
                                          in_=ctx_pairs[0])
                    # output projection + residual + LN for this chunk
                    for ts_ in range(4):
                        tt = 4 * c + ts_
                        ps_o = psB.tile([P, 512], F32, tag="b")
                        for d in range(ND):
                            nc.tensor.matmul(
                                ps_o, ctx_pairs[d][:, ts_ * P:(ts_ + 1) * P],
                                wo[:, d, :], start=(d == 0), stop=(d == ND - 1))
                        ln_into_xres(ps_o, tt)
                    # transpose this chunk's tiles now (overlaps with the
                    # other chunk's scores; xT fully consumed pre-c-loop)
                    transpose_to(xT, lambda tt: x_res[:, tt, :],
                                 list(range(4 * c, 4 * c + 4)))
                if DBG and l == 0 and is_self:
                    nc.sync.dma_start(out=dbg["dbg_xa"][:, :],
                                      in_=x_res.rearrange("p t d -> p (t d)"))

            def ffn(l, last):
                for c in range(2):
                    ps_fs = [psB.tile([P, 512], F32, tag="b",
                                      name=f"psf_{l}_{c}_{i}") for i in range(4)]
                    for j in range(NJ):
                        if j % 4 == 0:
                            jc = j // 4
                            w1t = dbl.tile([P, ND, 512], BF16, tag="w1c")
                            nc.gpsimd.dma_start(
                                out=w1t,
                                in_=W1D[l][:, jc * 512:(jc + 1) * 512]
                                .rearrange("(kc kp) n -> kp kc n", kp=P))
                            w2t = dbl.tile([P, 4, D], BF16, tag="w2c")
                            nc.gpsimd.dma_start(
                                out=w2t,
                                in_=W2D[l][jc * 512:(jc + 1) * 512, :]
                                .rearrange("(jj kp) n -> kp jj n", kp=P))
                        ps_h = psS.tile([P, 512], F32, tag="s")
                        for kc in range(ND):
                            nc.tensor.matmul(
                                ps_h, w1t[:, kc, (j % 4) * P:(j % 4 + 1) * P],
                                xT[:, kc, c * 512:(c + 1) * 512],
                                start=(kc == 0), stop=(kc == ND - 1))
                        hT = p3.tile([P, 512], BF16, tag="hT")
                        nc.scalar.activation(out=hT, in_=ps_h, func=AF.Relu)
                        for ts_ in range(4):
                            nc.tensor.matmul(
                                ps_fs[ts_], hT[:, ts_ * P:(ts_ + 1) * P],
                                w2t[:, j % 4, :],
                                start=(j == 0), stop=(j == NJ - 1))
                    for ts_ in range(4):
                        ln_into_xres(ps_fs[ts_], 4 * c + ts_)
                    if not last:
                        transpose_to(xT, lambda tt: x_res[:, tt, :],
                                     list(range(4 * c, 4 * c + 4)))

            # ---------------- layers ----------------
            for rep in range(repeat):
                for l in range(n_layers):
                    attn(l, True)
                    attn(l, False)
                    ffn(l, last=(rep == repeat - 1 and l == n_layers - 1))

            for tt in range(NT):
                nc.sync.dma_start(out=outd[tt * P:(tt + 1) * P, :],
                                  in_=x_res[:, tt, :])

    nc.compile()
    return nc


_CACHE = {}


def get_nc(n_layers=L_FULL, repeat=1):
    key = (n_layers, repeat)
    if key not in _CACHE:
        _CACHE[key] = build(n_layers, repeat)
    return _CACHE[key]


def make_in_maps(dec_inputs, enc_inputs, enc_outputs, emb,
                 Wq_self, Wk_self, Wv_self, Wo_self,
                 Wq_cross, Wk_cross, Wv_cross, Wo_cross, W1, W2,
                 n_layers=L_FULL):
    f = np.ascontiguousarray
    emb = f(np.asarray(emb, dtype=np.float32))
    dec = np.asarray(dec_inputs).astype(np.int32)
    enc = np.asarray(enc_inputs).astype(np.int32)
    encx = np.asarray(enc_outputs, dtype=np.float32)
    pe = _pe_table()
    caus = _causal_quads().astype(BF)
    identr = np.eye(P, dtype=BF)
    ident = np.eye(P, dtype=np.float32)
    ws = {}
    for name, w in (("wq_s", Wq_self), ("wk_s", Wk_self), ("wv_s", Wv_self),
                    ("wo_s", Wo_self), ("wq_c", Wq_cross), ("wk_c", Wk_cross),
                    ("wv_c", Wv_cross), ("wo_c", Wo_cross), ("w1", W1),
                    ("w2", W2)):
        ws[name] = f(np.asarray(w, dtype=np.float32)[:n_layers].astype(BF))
    B = dec.shape[0]
    in_maps = []
    for b in range(B):
        m = dict(emb=emb, pe=pe, causal=caus, ident=ident, identr=identr,
                 dec_idx=f(dec[b].reshape(NT, P).T),
                 enc_idx=f(enc[b].reshape(NT, P).T),
                 encx=f(encx[b]), **ws)
        in_maps.append(m)
    return in_maps


LAST_RESULT = None


def kernel(**inputs):
    global LAST_RESULT
    n_layers = inputs.pop("_n_layers", L_FULL)
    trace = inputs.pop("_trace", False)
    tmpdir = inputs.pop("_tmpdir", None)
    nc = get_nc(n_layers)
    in_maps = make_in_maps(**inputs, n_layers=n_layers)
    res = run_bass_kernel_spmd(nc, in_maps, core_ids=list(range(len(in_maps))),
                               trace=trace, tmpdir=tmpdir)
    LAST_RESULT = res
    out = np.stack([r["out"] for r in res.results], axis=0)
    return out


# revision 30
# speedup vs baseline: 1.4579x; 1.0566x over previous
import os
import sys
sys.path.insert(0, '/opt/trn_rl_repo')
import numpy as np
import ml_dtypes
import concourse.bass as bass
import concourse.bacc as bacc
import concourse.mybir as mybir
import concourse.tile as tile
from concourse.bass import IndirectOffsetOnAxis
from concourse.bass_utils import run_bass_kernel_spmd

P = 128
T = 1024
S = 1024
D = 512
H = 8
DK = 64
DFF = 2048
VOC = 32000
NT = T // P   # 8 token tiles
ND = D // P   # 4 d-model chunks
NJ = DFF // P  # 16 dff tiles
L_FULL = 6
EPS = 1e-5
NEG = -1e9

F32 = mybir.dt.float32
BF16 = mybir.dt.bfloat16
I32 = mybir.dt.int32
AF = mybir.ActivationFunctionType
OP = mybir.AluOpType
BF = ml_dtypes.bfloat16


def _pe_table():
    pos = np.arange(T)[:, None].astype(np.float64)
    div = np.exp(np.arange(0, D, 2).astype(np.float64) * (-np.log(10000.0) / D))
    pe = np.zeros((T, D))
    pe[:, 0::2] = np.sin(pos * div)
    pe[:, 1::2] = np.cos(pos * div)
    return pe.astype(np.float32)


def _causal_quads():
    # [P, 4, 512] multiplicative mask: 0 where (128*r + kk) > qq else 1
    kk = np.arange(P)[:, None]
    qq = np.arange(512)[None, :]
    out = np.zeros((P, 4, 512), np.float32)
    for r in range(4):
        out[:, r, :] = np.where(128 * r + kk > qq, 0.0, 1.0)
    return out


def build(n_layers=L_FULL, repeat=1):
    DBG = os.environ.get("KDBG", "0") == "1"
    nc = bacc.Bacc("TRN2", target_bir_lowering=False, debug=False, num_devices=8)

    embd = nc.dram_tensor("emb", [VOC, D], F32, kind="ExternalInput")
    decd = nc.dram_tensor("dec_idx", [P, NT], I32, kind="ExternalInput")
    encd = nc.dram_tensor("enc_idx", [P, NT], I32, kind="ExternalInput")
    ped = nc.dram_tensor("pe", [T, D], F32, kind="ExternalInput")
    causd = nc.dram_tensor("causal", [P, 4, 512], BF16, kind="ExternalInput")
    identd = nc.dram_tensor("ident", [P, P], BF16, kind="ExternalInput")
    encxd = nc.dram_tensor("encx", [S, D], F32, kind="ExternalInput")
    WQS = nc.dram_tensor("wq_s", [n_layers, D, D], BF16, kind="ExternalInput")
    WKS = nc.dram_tensor("wk_s", [n_layers, D, D], BF16, kind="ExternalInput")
    WVS = nc.dram_tensor("wv_s", [n_layers, D, D], BF16, kind="ExternalInput")
    WOS = nc.dram_tensor("wo_s", [n_layers, D, D], BF16, kind="ExternalInput")
    WQC = nc.dram_tensor("wq_c", [n_layers, D, D], BF16, kind="ExternalInput")
    WKC = nc.dram_tensor("wk_c", [n_layers, D, D], BF16, kind="ExternalInput")
    WVC = nc.dram_tensor("wv_c", [n_layers, D, D], BF16, kind="ExternalInput")
    WOC = nc.dram_tensor("wo_c", [n_layers, D, D], BF16, kind="ExternalInput")
    W1D = nc.dram_tensor("w1", [n_layers, D, DFF], BF16, kind="ExternalInput")
    W2D = nc.dram_tensor("w2", [n_layers, DFF, D], BF16, kind="ExternalInput")
    outd = nc.dram_tensor("out", [T, D], F32, kind="ExternalOutput")
    if DBG:
        dbg = {
            "dbg_x0": nc.dram_tensor("dbg_x0", [P, NT * D], F32,
                                     kind="ExternalOutput"),
            "dbg_xT": nc.dram_tensor("dbg_xT", [P, ND * T], BF16,
                                     kind="ExternalOutput"),
            "dbg_QT": nc.dram_tensor("dbg_QT", [P, ND * T], BF16,
                                     kind="ExternalOutput"),
            "dbg_KT": nc.dram_tensor("dbg_KT", [P, ND * T], BF16,
                                     kind="ExternalOutput"),
            "dbg_vext": nc.dram_tensor("dbg_vext", [P, NT * H * P],
                                       BF16, kind="ExternalOutput"),
            "dbg_e": nc.dram_tensor("dbg_e", [P, 1024], BF16,
                                    kind="ExternalOutput"),
            "dbg_recip": nc.dram_tensor("dbg_recip", [1, 512], F32,
                                        kind="ExternalOutput"),
            "dbg_rb": nc.dram_tensor("dbg_rb", [DK, 512], F32,
                                     kind="ExternalOutput"),
            "dbg_ctx": nc.dram_tensor("dbg_ctx", [P, 512], BF16,
                                      kind="ExternalOutput"),
            "dbg_xa": nc.dram_tensor("dbg_xa", [P, NT * D], F32,
                                     kind="ExternalOutput"),
        }

    with nc.allow_low_precision(reason="bf16 matmuls intended"), \
         tile.TileContext(nc) as tc:
        with tc.tile_pool(name="pers", bufs=1) as pers, \
             tc.tile_pool(name="dbl", bufs=2) as dbl, \
             tc.tile_pool(name="p3", bufs=3) as p3, \
             tc.tile_pool(name="p2", bufs=4) as p2, \
             tc.tile_pool(name="p5", bufs=5) as p5, \
             tc.tile_pool(name="p4", bufs=4) as p4, \
             tc.tile_pool(name="psS", bufs=2, space="PSUM") as psS, \
             tc.tile_pool(name="psB", bufs=4, space="PSUM") as psB:

            # ---------------- persistent tiles ----------------
            x_res = pers.tile([P, NT, D], F32)       # [tok_in_tile, t_tile, D]
            xT = pers.tile([P, ND, T], BF16)         # [d_in_chunk, d_chunk, tok]
            encT = pers.tile([P, ND, S], BF16)
            QT = pers.tile([P, ND, T], BF16)
            KT = pers.tile([P, ND, T], BF16)
            # per (tile, head): col 0 = ones (pad-masked), cols 64..127 = V
            vext = pers.tile([P, NT, H, P], BF16)
            causal_sb = pers.tile([P, 4, 512], BF16)
            ident_sb = pers.tile([P, P], BF16)
            eps_sb = pers.tile([P, 1], F32)
            dec_sb = pers.tile([P, NT], I32)
            enc_sb = pers.tile([P, NT], I32)
            msc_dec = pers.tile([P, NT], F32)
            msc_enc = pers.tile([P, NT], F32)

            nc.sync.dma_start(out=causal_sb, in_=causd[:, :, :])
            nc.sync.dma_start(out=ident_sb, in_=identd[:, :])
            nc.sync.dma_start(out=dec_sb, in_=decd[:, :])
            nc.sync.dma_start(out=enc_sb, in_=encd[:, :])
            nc.vector.memset(eps_sb, EPS)
            nc.vector.memset(vext, 0.0)

            # pad multipliers (0 for pad token, 1 otherwise)
            for tok_sb, msc in ((dec_sb, msc_dec), (enc_sb, msc_enc)):
                tokf = p4.tile([P, NT], F32, tag="tokf")
                nc.vector.tensor_copy(out=tokf, in_=tok_sb)
                is0 = p4.tile([P, NT], F32, tag="is0")
                nc.vector.tensor_scalar(out=is0, in0=tokf, scalar1=0.0,
                                        scalar2=None, op0=OP.is_equal)
                nc.scalar.activation(out=msc, in_=is0, func=AF.Copy,
                                     bias=1.0, scale=-1.0)

            def transpose_to(dst, src_bf_of_tt, tts):
                # dst [P, ND, T] bf16; src_bf_of_tt(tt) -> [P, D] bf16 AP.
                for d in range(ND):
                    for tt in tts:
                        ps_t = psB.tile([P, P], BF16, tag="b",
                                        name=f"pst_{d}_{tt}")
                        nc.tensor.transpose(
                            out=ps_t,
                            in_=src_bf_of_tt(tt)[:, d * P:(d + 1) * P],
                            identity=ident_sb)
                        nc.vector.tensor_copy(
                            out=dst[:, d, tt * P:(tt + 1) * P], in_=ps_t)

            def transpose_xres_to_xT(tts):
                # cast f32 x_res tiles to bf16, then transpose into xT
                for tt in tts:
                    xb = p4.tile([P, D], BF16, tag="xbf", name=f"xbf_{tt}")
                    nc.vector.tensor_copy(out=xb, in_=x_res[:, tt, :])
                    for d in range(ND):
                        ps_t = psB.tile([P, P], BF16, tag="b",
                                        name=f"pstx_{d}_{tt}")
                        nc.tensor.transpose(out=ps_t,
                                            in_=xb[:, d * P:(d + 1) * P],
                                            identity=ident_sb)
                        nc.vector.tensor_copy(
                            out=xT[:, d, tt * P:(tt + 1) * P], in_=ps_t)

            # enc transposes first: PE work available while embedding runs
            for g0 in range(0, NT, 4):
                e_ts = []
                for tt in range(g0, g0 + 4):
                    e_t = p4.tile([P, D], BF16, tag="enc", name=f"enc_{tt}")
                    e_f = p4.tile([P, D], F32, tag="encf", name=f"encf_{tt}")
                    nc.sync.dma_start(out=e_f, in_=encxd[tt * P:(tt + 1) * P, :])
                    nc.vector.tensor_copy(out=e_t, in_=e_f)
                    e_ts.append(e_t)
                transpose_to(encT, lambda tt: e_ts[tt - g0],
                             list(range(g0, g0 + 4)))

            # ---------------- embedding + pe ----------------
            for tt in range(NT):
                g = p3.tile([P, D], F32, tag="tmp")
                nc.gpsimd.indirect_dma_start(
                    out=g, out_offset=None, in_=embd[:, :],
                    in_offset=IndirectOffsetOnAxis(ap=dec_sb[:, tt:tt + 1], axis=0))
                pe_t = p3.tile([P, D], F32, tag="tmp")
                nc.sync.dma_start(out=pe_t, in_=ped[tt * P:(tt + 1) * P, :])
                g2 = p3.tile([P, D], F32, tag="tmp")
                nc.vector.tensor_scalar(out=g2, in0=g, scalar1=msc_dec[:, tt:tt + 1],
                                        scalar2=None, op0=OP.mult)
                nc.vector.tensor_add(out=x_res[:, tt, :], in0=g2, in1=pe_t)
            transpose_xres_to_xT(list(range(NT)))
            if DBG:
                nc.sync.dma_start(out=dbg["dbg_x0"][:, :],
                                  in_=x_res.rearrange("p t d -> p (t d)"))
                nc.sync.dma_start(out=dbg["dbg_xT"][:, :],
                                  in_=xT.rearrange("p c t -> p (c t)"))

            # ---------------- helpers ----------------
            def ln_into_xres(ps_in, tt):
                pre = p3.tile([P, D], F32, tag="tmp")
                nc.vector.tensor_add(out=pre, in0=ps_in, in1=x_res[:, tt, :])
                st = p4.tile([P, nc.vector.BN_STATS_DIM], F32, tag="st")
                nc.vector.bn_stats(out=st, in_=pre)
                mv = p4.tile([P, nc.vector.BN_AGGR_DIM], F32, tag="mv")
                nc.vector.bn_aggr(out=mv, in_=st)
                std = p4.tile([P, 1], F32, tag="sd")
                nc.scalar.activation(out=std, in_=mv[:, 1:2], func=AF.Sqrt,
                                     bias=eps_sb, scale=1.0)
                rstd = p4.tile([P, 1], F32, tag="rs")
                nc.vector.reciprocal(out=rstd, in_=std)
                nc.vector.tensor_scalar(out=x_res[:, tt, :], in0=pre,
                                        scalar1=mv[:, 0:1], scalar2=rstd,
                                        op0=OP.subtract, op1=OP.mult)

            def load_wattn(wd, l):
                w = dbl.tile([P, ND, D], BF16, tag="wattn")
                nc.gpsimd.dma_start(
                    out=w, in_=wd[l].rearrange("(kc kp) n -> kp kc n", kp=P))
                return w

            def attn(l, is_self):
                wq = load_wattn(WQS if is_self else WQC, l)
                wk = load_wattn(WKS if is_self else WKC, l)
                wv = load_wattn(WVS if is_self else WVC, l)
                wo = load_wattn(WOS if is_self else WOC, l)
                kv = xT if is_self else encT
                msc = msc_dec if is_self else msc_enc

                # QT / KT projections
                for dst, w, src in ((QT, wq, xT), (KT, wk, kv)):
                    for dq in range(ND):
                        ps = psS.tile([P, 1024], F32, tag="s")
                        for kc in range(ND):
                            for c in range(2):
                                nc.tensor.matmul(
                                    ps[:, c * 512:(c + 1) * 512],
                                    w[:, kc, dq * P:(dq + 1) * P],
                                    src[:, kc, c * 512:(c + 1) * 512],
                                    start=(kc == 0), stop=(kc == ND - 1),
                                    skip_group_check=True)
                        nc.any.tensor_copy(out=dst[:, dq, :], in_=ps)

                # V projection, pad rows zeroed via msc
                for i in range(NT):
                    ps = psS.tile([P, 512], F32, tag="s")
                    for kc in range(ND):
                        nc.tensor.matmul(ps, kv[:, kc, i * P:(i + 1) * P],
                                         wv[:, kc, :],
                                         start=(kc == 0), stop=(kc == ND - 1))
                    nc.vector.tensor_scalar(
                        out=vext[:, i, :, DK:P],
                        in0=ps.rearrange("p (h v) -> p h v", h=H),
                        scalar1=msc[:, i:i + 1],
                        scalar2=None, op0=OP.mult)
                # ones column (also zeroed on pad rows)
                for h in range(H):
                    nc.vector.tensor_copy(
                        out=vext[:, :, h, 0:1],
                        in_=msc.rearrange("p (t o) -> p t o", o=1))
                if DBG and l == 0 and is_self:
                    nc.sync.dma_start(out=dbg["dbg_QT"][:, :],
                                      in_=QT.rearrange("p c t -> p (c t)"))
                    nc.sync.dma_start(out=dbg["dbg_KT"][:, :],
                                      in_=KT.rearrange("p c t -> p (c t)"))
                    nc.sync.dma_start(
                        out=dbg["dbg_vext"][:, :],
                        in_=vext.rearrange("p t h v -> p (t h v)"))

                # scores -> exp -> AV (k-tiles processed in pairs)
                for c in range(2):
                    ctx_pairs = [p5.tile([P, 512], BF16, tag="ctx",
                                         name=f"ctxp_{l}_{is_self}_{c}_{d}")
                                 for d in range(ND)]
                    for d in range(ND):
                        for hh in range(2):
                            h = 2 * d + hh
                            hsl = slice(hh * 64, (hh + 1) * 64)
                            kmax = 4 * (c + 1) if is_self else NT
                            ps_ctx = psB.tile([P, 512], F32, tag="b")
                            for i0 in range(0, kmax, 2):
                                ps_s = psS.tile([P, 1024], F32, tag="s")
                                diag = is_self and i0 >= 4 * c
                                for half, i in ((0, i0), (1, i0 + 1)):
                                    sl_ = slice(half * 512, (half + 1) * 512)
                                    nc.tensor.matmul(
                                        ps_s[:, sl_],
                                        KT[hsl, d, i * P:(i + 1) * P],
                                        QT[hsl, d, c * 512:(c + 1) * 512],
                                        start=True, stop=True,
                                        skip_group_check=True)
                                e = p2.tile([P, 1024], BF16, tag="exp")
                                nc.scalar.activation(out=e, in_=ps_s,
                                                     func=AF.Exp, scale=0.125)
                                if diag:
                                    # zero the causally-masked exp weights
                                    # (bf16 mult hits the DVE 4x fast path)
                                    r0 = i0 - 4 * c
                                    em = p2.tile([P, 1024], BF16, tag="exp")
                                    nc.vector.tensor_mul(
                                        out=em, in0=e,
                                        in1=causal_sb[:, r0:r0 + 2, :]
                                        .rearrange("p r q -> p (r q)"))
                                    e = em
                                if (DBG and l == 0 and is_self and c == 0
                                        and d == 0 and hh == 0 and i0 == 0):
                                    nc.sync.dma_start(out=dbg["dbg_e"][:, :], in_=e)
                                for half, i in ((0, i0), (1, i0 + 1)):
                                    nc.tensor.matmul(
                                        ps_ctx, vext[:, i, h, :],
                                        e[:, half * 512:(half + 1) * 512],
                                        start=(i == 0), stop=(i == kmax - 1))
                            recip = p3.tile([1, 512], F32, tag="recip")
                            nc.vector.reciprocal_approx_fast(
                                out=recip, in_=ps_ctx[0:1, :])
                            rb = p3.tile([DK, 512], F32, tag="rb")
                            nc.gpsimd.partition_broadcast(rb, recip)
                            nc.vector.tensor_mul(out=ctx_pairs[d][hsl, :],
                                                 in0=ps_ctx[DK:P, :], in1=rb)
                            if (DBG and l == 0 and is_self and c == 0
                                    and d == 0 and hh == 0):
                                nc.sync.dma_start(out=dbg["dbg_recip"][:, :],
                                                  in_=recip)
                                nc.sync.dma_start(out=dbg["dbg_rb"][:, :], in_=rb)
                    if DBG and l == 0 and is_self and c == 0:
                        nc.sync.dma_start(out=dbg["dbg_ctx"][:, :],
                                          in_=ctx_pairs[0])
                    # output projection + residual + LN for this chunk
                    for ts_ in range(4):
                        tt = 4 * c + ts_
                        ps_o = psB.tile([P, 512], F32, tag="b")
                        for d in range(ND):
                            nc.tensor.matmul(
                                ps_o, ctx_pairs[d][:, ts_ * P:(ts_ + 1) * P],
                                wo[:, d, :], start=(d == 0), stop=(d == ND - 1))
                        ln_into_xres(ps_o, tt)
                    # transpose this chunk's tiles now (overlaps with the
                    # other chunk's scores; xT fully consumed pre-c-loop)
                    transpose_xres_to_xT(list(range(4 * c, 4 * c + 4)))
                if DBG and l == 0 and is_self:
                    nc.sync.dma_start(out=dbg["dbg_xa"][:, :],
                                      in_=x_res.rearrange("p t d -> p (t d)"))

            def ffn(l, last):
                for c in range(2):
                    ps_fs = [psB.tile([P, 512], F32, tag="b",
                                      name=f"psf_{l}_{c}_{i}") for i in range(4)]
                    for j in range(NJ):
                        if j % 4 == 0:
                            jc = j // 4
                            w1t = dbl.tile([P, ND, 512], BF16, tag="w1c")
                            nc.gpsimd.dma_start(
                                out=w1t,
                                in_=W1D[l][:, jc * 512:(jc + 1) * 512]
                                .rearrange("(kc kp) n -> kp kc n", kp=P))
                            w2t = dbl.tile([P, 4, D], BF16, tag="w2c")
                            nc.gpsimd.dma_start(
                                out=w2t,
                                in_=W2D[l][jc * 512:(jc + 1) * 512, :]
                                .rearrange("(jj kp) n -> kp jj n", kp=P))
                        ps_h = psS.tile([P, 512], F32, tag="s")
                        for kc in range(ND):
                            nc.tensor.matmul(
                                ps_h, w1t[:, kc, (j % 4) * P:(j % 4 + 1) * P],
                                xT[:, kc, c * 512:(c + 1) * 512],
                                start=(kc == 0), stop=(kc == ND - 1))
                        hT = p3.tile([P, 512], BF16, tag="hT")
                        nc.vector.tensor_scalar(out=hT, in0=ps_h, scalar1=0.0,
                                                scalar2=None, op0=OP.max)
                        for ts_ in range(4):
                            nc.tensor.matmul(
                                ps_fs[ts_], hT[:, ts_ * P:(ts_ + 1) * P],
                                w2t[:, j % 4, :],
                                start=(j == 0), stop=(j == NJ - 1))
                    for ts_ in range(4):
                        ln_into_xres(ps_fs[ts_], 4 * c + ts_)
                    if not last:
                        transpose_xres_to_xT(list(range(4 * c, 4 * c + 4)))

            # ---------------- layers ----------------
            for rep in range(repeat):
                for l in range(n_layers):
                    attn(l, True)
                    attn(l, False)
                    ffn(l, last=(rep == repeat - 1 and l == n_layers - 1))

            for tt in range(NT):
                nc.sync.dma_start(out=outd[tt * P:(tt + 1) * P, :],
                                  in_=x_res[:, tt, :])

    nc.compile()
    return nc


_CACHE = {}


def get_nc(n_layers=L_FULL, repeat=1):
    key = (n_layers, repeat)
    if key not in _CACHE:
        _CACHE[key] = build(n_layers, repeat)
    return _CACHE[key]


def make_in_maps(dec_inputs, enc_inputs, enc_outputs, emb,
                 Wq_self, Wk_self, Wv_self, Wo_self,
                 Wq_cross, Wk_cross, Wv_cross, Wo_cross, W1, W2,
                 n_layers=L_FULL):
    f = np.ascontiguousarray
    emb = f(np.asarray(emb, dtype=np.float32))
    dec = np.asarray(dec_inputs).astype(np.int32)
    enc = np.asarray(enc_inputs).astype(np.int32)
    encx = np.asarray(enc_outputs, dtype=np.float32)
    pe = _pe_table()
    caus = _causal_quads().astype(BF)
    ident = np.eye(P, dtype=BF)
    ws = {}
    for name, w in (("wq_s", Wq_self), ("wk_s", Wk_self), ("wv_s", Wv_self),
                    ("wo_s", Wo_self), ("wq_c", Wq_cross), ("wk_c", Wk_cross),
                    ("wv_c", Wv_cross), ("wo_c", Wo_cross), ("w1", W1),
                    ("w2", W2)):
        ws[name] = f(np.asarray(w, dtype=np.float32)[:n_layers].astype(BF))
    B = dec.shape[0]
    in_maps = []
    for b in range(B):
        m = dict(emb=emb, pe=pe, causal=caus, ident=ident,
                 dec_idx=f(dec[b].reshape(NT, P).T),
                 enc_idx=f(enc[b].reshape(NT, P).T),
                 encx=f(encx[b]), **ws)
        in_maps.append(m)
    return in_maps


LAST_RESULT = None


def kernel(**inputs):
    global LAST_RESULT
    n_layers = inputs.pop("_n_layers", L_FULL)
    trace = inputs.pop("_trace", False)
    tmpdir = inputs.pop("_tmpdir", None)
    nc = get_nc(n_layers)
    in_maps = make_in_maps(**inputs, n_layers=n_layers)
    res = run_bass_kernel_spmd(nc, in_maps, core_ids=list(range(len(in_maps))),
                               trace=trace, tmpdir=tmpdir)
    LAST_RESULT = res
    out = np.stack([r["out"] for r in res.results], axis=0)
    return out
